# revision 20
# baseline (speedup 1.0000x reference)
"""Bass/Trainium2 kernel for nn_Attn_13846974562399.

Reference computation:
    proj   = enc @ W^T + bias          # [S, B, H]
    scores = einsum('bh,sbh->bs', hidden[0], proj)
    attn   = softmax(scores, axis=1)   # -> [B, 1, S]

Algebraic restructure:
    scores[b, s] = q[b] . enc[s, b],   q = hidden[0] @ W
(the hidden.bias term is constant over s and cancels in softmax).  q is
computed on the host in float64; the memory-bound work -- streaming the
encoder tensor and the S*B*H dot-product contraction -- runs on 8
NeuronCores, data-parallel over batch (4 local batches per core).

Precision strategy (the memory-regime key move).  The harness gate is
rel_err < 2e-2.  The device streams the encoder in FP8 E4M3 (8.4 MB per
core, 4x less than fp32) and computes approximate scores s~ = q8 . enc8
with fp32 accumulation; per-score error is ~N(0, 1.2^2).  The host then
(1) ranks each row by s~ and recomputes the top-64 scores EXACTLY
(float64 q . enc from the original fp32 input; 64*H MACs per row =
0.002% of the device FLOPs), and (2) applies softmax over {exact
top-64, fp8 tail}.  Score rows are extremely peaked (std ~32 over 2048
entries), so the tail mass beyond the top-64 is ~1e-13 of the total and
its fp8 distortion is irrelevant: end-to-end rel err measured on
hardware is ~5e-6 (fp16-everywhere gives 6e-3; fp8 without refinement
fails).  Ranking is safe: a true-top entry would need a -10-sigma fp8
error to be misranked out of 64.

Device program (per core).  With the stream at fp8 the DMA is ~26 us
busy (16 SDMA engines, byte-bound) and a single compute engine becomes
the critical path -- a PE-only version measured 259 ns per
[128x1]x[128,512] matmul (~34 us chain; fp8 DoubleRow mode, which would
halve that, crashes this NEFF backend).  So the s-range is SPLIT across
two engines, each with the layout that suits it:

- s in [0, 1408) (68.75%): Tensor engine.  Host layout [b, hc, p, s]
  (h = hc*128+p, contraction dim h on partitions); transfers of
  [128, 1408] fp8 per (b, hc).  Three matmuls per transfer with
  1-column stationary weights accumulate the 8 h-chunks of each score
  group in fp32 PSUM (s-tiles 512/512/384; a PSUM-bank-crossing matmul
  out crashes the backend).  ~23 us.
- s in [1408, 2048) (31.25%): Vector engine (otherwise idle; 8-bit STT
  runs 1 elem/lane/cycle at 0.96 GHz).  Host layout [b, tp, p, t2, h]
  with s = 1408 + (2*tp+t2)*128 + p (s on partitions, t-PAIRS per
  transfer for 2 KB partition lines); one fused scalar_tensor_tensor
  per [128, 1024] chunk multiplies by a replicated q row tile and
  reduces over h into a [128, 20] f32 score tile.  ~24 us.  (Verified
  on HW: fp8 STT inputs with f32 accum_out, rel err 7e-8.)

Transfers are issued in per-batch need order (a DVE pair-chunk feeds
~2.4 us of STT, a PE transfer ~0.73 us of matmul) and each goes to
whichever HWDGE ring has fewer cumulative bytes -- plain alternation
left one ring ~50% heavier inside each batch window and the PE starved
mid-stream for ~6 us waiting on the heavy ring.  Every transfer owns a
private SBUF buffer (~9 MB) so the stream never waits on compute.
~8 warm-up matmuls spin the PE clock from 0.65 toward 2.4 GHz before
real data lands.  Raw fp32 scores ship to the host: PE scores via
PSUM->SBUF copies (split across the scalar AND vector engines -- a
serial 3-copy chain on one engine added ~1.7 us to the tail) then
per-batch DMA (gpsimd SWDGE queue mid-stream -- a dependent trigger on
an in-order HWDGE ring sequencer parks the whole ring -- and the idle
rings for the last batch); DVE scores as one [128, 20] tile at the
end.  No exp/normalization on device -- softmax happens in the host
refinement step.  PSUM: one 3-bank [128, 1536] tile per batch pair,
batch b at base partition 32*(b%2) (PE tile_position allows out base
partitions {0, 32, 64}).
"""

import numpy as np
import ml_dtypes

import concourse.bacc as bacc
import concourse.bass as bass
import concourse.mybir as mybir
import concourse.tile as tile
from concourse.bass_utils import run_bass_kernel_spmd

S, B, H = 2048, 32, 1024
NCORES = 8
BL = B // NCORES          # 4 local batches per core
P = 128                   # SBUF partitions
HC = H // P               # 8 h-chunks per batch
SP = 1408                 # s in [0, SP) on the PE
SD = S - SP               # s in [SP, S) on the DVE
NT = SD // P              # 5 DVE 128-s chunks per batch
NTP = (NT + 1) // 2       # 3 DVE transfers per batch (2+2+1 chunks)
PE_TILES = (512, 512, 384)
F8 = mybir.dt.float8e4
F32 = mybir.dt.float32
NP8 = ml_dtypes.float8_e4m3

WARMUP_MM = 8             # PE pstate ramp matmuls before the stream
TOPK = 64                 # host-refined candidates per row

LAST_RESULTS = None
TRACE = False

_NC = None


def _build_bass():
    nc = bacc.Bacc()
    ence = nc.dram_tensor("ence", [BL, HC, P, SP], F8, kind="ExternalInput")
    # DVE share padded to 6 chunks (3 pair-transfers); chunk 5 is junk.
    encd = nc.dram_tensor("encd", [BL, NTP, P, 2, H], F8, kind="ExternalInput")
    qw = nc.dram_tensor("qw", [P, BL * HC], F8, kind="ExternalInput")
    qrep = nc.dram_tensor("qrep", [BL, P, H], F8, kind="ExternalInput")
    out = nc.dram_tensor("sc", [BL, SP], F32, kind="ExternalOutput")
    outd = nc.dram_tensor("dsc", [P, BL * NT], F32, kind="ExternalOutput")

    mult = mybir.AluOpType.mult

    with tile.TileContext(nc) as tc:
        with (
            tc.tile_pool(name="encp", bufs=BL * HC) as enc_pool,
            tc.tile_pool(name="dvep", bufs=BL * NTP) as dve_pool,
            tc.tile_pool(name="small", bufs=1) as small,
            tc.psum_pool(name="pp", bufs=1) as pp,
        ):
            qw_sb = small.tile([P, BL * HC], F8)
            qrep_sb = [small.tile([P, H], F8, name=f"qr{b}") for b in range(BL)]
            s_sb = small.tile([P, SP], F32)
            dsc = small.tile([P, BL * NT], F32)
            dummy = small.tile([P, 1], F32)
            warm = small.tile([P, 512], F8)

            psum_t = [pp.tile([P, 1536], F32, name=f"ps{g}") for g in range(BL // 2)]

            ence_ap = ence.ap()
            encd_ap = encd.ap()
            out_ap = out.ap()

            ring_bytes = [0, 0]
            rings = [nc.sync, nc.scalar]

            def ring(nbytes):
                i = 0 if ring_bytes[0] <= ring_bytes[1] else 1
                ring_bytes[i] += nbytes
                return rings[i]

            # Per-batch transfer schedule in engine need order; each
            # transfer goes to the ring with fewer cumulative bytes.
            pe_t = {}
            dve_t = {}
            first_issued = False
            ORDER = ["d0", "h0", "h1", "h2", "d1", "h3", "h4", "h5", "d2", "h6", "h7"]
            for b in range(BL):
                for item in ORDER:
                    k = int(item[1])
                    if item[0] == "h":
                        et = enc_pool.tile([P, SP], F8)
                        ring(P * SP).dma_start(out=et, in_=ence_ap[b, k])
                        pe_t[(b, k)] = et
                    else:
                        npairs = 2 if k < NTP - 1 else NT - 2 * (NTP - 1)
                        dt = dve_pool.tile([P, 2, H], F8)
                        if npairs == 2:
                            ring(P * 2 * H).dma_start(out=dt, in_=encd_ap[b, k])
                        else:
                            ring(P * H).dma_start(
                                out=dt[:, 0, :], in_=encd_ap[b, k, :, 0]
                            )
                        dve_t[(b, k)] = dt
                    if not first_issued:
                        # The small q tiles slot in right after the first
                        # two stream transfers on both rings.
                        first_issued = True
                        nc.scalar.dma_start(out=qw_sb, in_=qw.ap())
                        for bb in range(BL):
                            nc.sync.dma_start(
                                out=qrep_sb[bb], in_=qrep.ap()[bb]
                            )

            nc.vector.memset(warm, 0.0)
            for _ in range(WARMUP_MM):
                nc.tensor.matmul(
                    out=psum_t[0][64:65, 0:512],
                    lhsT=warm[:, 0:1],
                    rhs=warm,
                    start=True,
                    stop=True,
                )

            for b in range(BL):
                row = slice(32 * (b % 2), 32 * (b % 2) + 1)
                srow = slice(32 * b, 32 * b + 1)
                ps = psum_t[b // 2]
                # DVE chunks for this batch (independent of the PE path).
                for t in range(NT):
                    dt = dve_t[(b, t // 2)]
                    nc.vector.scalar_tensor_tensor(
                        out=dummy.broadcast_to((P, H)),
                        in0=dt[:, t % 2, :],
                        scalar=1.0,
                        in1=qrep_sb[b][:],
                        op0=mult,
                        op1=mult,
                        accum_out=dsc[:, b * NT + t : b * NT + t + 1],
                    )
                # PE h-chunk accumulation.
                for hc in range(HC):
                    c = b * HC + hc
                    et = pe_t[(b, hc)]
                    off = 0
                    for w in PE_TILES:
                        nc.tensor.matmul(
                            out=ps[row, off : off + w],
                            lhsT=qw_sb[:, c : c + 1],
                            rhs=et[:, off : off + w],
                            start=(hc == 0),
                            stop=(hc == HC - 1),
                        )
                        off += w
                # Copies split across scalar+vector so the tail chain is
                # parallel, then one DMA per batch.
                nc.scalar.copy(out=s_sb[srow, 0:512], in_=ps[row, 0:512])
                nc.vector.tensor_scalar_mul(
                    out=s_sb[srow, 512:1024], in0=ps[row, 512:1024], scalar1=1.0
                )
                nc.scalar.copy(out=s_sb[srow, 1024:1408], in_=ps[row, 1024:1408])
                if b < BL - 1:
                    nc.gpsimd.dma_start(out=out_ap[b], in_=s_sb[srow, :])
                else:
                    nc.sync.dma_start(out=out_ap[b], in_=s_sb[srow, :])
            nc.scalar.dma_start(out=outd.ap(), in_=dsc)

    nc.compile()
    return nc


def kernel(hidden, encoder_outputs, W, b):
    global _NC, LAST_RESULTS
    hidden = np.asarray(hidden, dtype=np.float32)
    enc = np.asarray(encoder_outputs, dtype=np.float32)
    W = np.asarray(W, dtype=np.float32)

    # q = hidden[0] @ W (fp64 accumulate on host).  The bias adds a per-b
    # constant to the scores, which softmax cancels, so `b` is unused.
    q64 = hidden[0].astype(np.float64) @ W.astype(np.float64)
    q8 = q64.astype(np.float32).astype(NP8)             # [B, H] fp8

    enc8 = enc.astype(NP8)                              # [S, B, H] fp8
    in_maps = []
    for c in range(NCORES):
        sl = enc8[:, BL * c : BL * (c + 1), :]          # [S, BL, H]
        # PE share: [b, h, s<SP] contiguous, h split as (hc, p).
        ence_r = np.ascontiguousarray(sl[:SP].transpose(1, 2, 0)).reshape(
            BL, HC, P, SP
        )
        # DVE share: [b, tp, p, t2, h] with s = SP + (2*tp+t2)*128 + p,
        # padded to 6 chunks (the 6th is junk, never transferred).
        dpad = np.zeros((2 * NTP, P, BL, H), dtype=NP8)
        dpad[:NT] = sl[SP:].reshape(NT, P, BL, H)
        encd_r = np.ascontiguousarray(
            dpad.reshape(NTP, 2, P, BL, H).transpose(3, 0, 2, 1, 4)
        )
        q_c = q8[BL * c : BL * (c + 1)]                 # [BL, H]
        qw_c = np.ascontiguousarray(
            q_c.reshape(BL, HC, P).transpose(2, 0, 1).reshape(P, BL * HC)
        )
        qrep_c = np.ascontiguousarray(
            np.broadcast_to(q_c[:, None, :], (BL, P, H))
        )
        in_maps.append(
            {"ence": ence_r, "encd": encd_r, "qw": qw_c, "qrep": qrep_c}
        )

    if _NC is None:
        _NC = _build_bass()

    LAST_RESULTS = run_bass_kernel_spmd(
        _NC, in_maps, core_ids=list(range(NCORES)), trace=TRACE
    )

    # Host epilogue: reassemble scores, rank rows by the device's fp8
    # scores, recompute the top-64 exactly, softmax over {exact top,
    # fp8 tail}.
    out = np.empty((B, 1, S), dtype=np.float32)
    for c in range(NCORES):
        res = LAST_RESULTS.results[c]
        sc = np.empty((BL, S), dtype=np.float64)
        sc[:, :SP] = res["sc"].astype(np.float64)
        # dsc[p, b*NT + t] -> s = SP + t*128 + p
        d = res["dsc"].astype(np.float64).reshape(P, BL, NT)
        sc[:, SP:] = d.transpose(1, 2, 0).reshape(BL, SD)
        for lb in range(BL):
            gb = BL * c + lb
            idx = np.argpartition(-sc[lb], TOPK)[:TOPK]
            sc[lb, idx] = enc[idx, gb, :].astype(np.float64) @ q64[gb]
            m = sc[lb].max()
            e = np.exp(sc[lb] - m)
            out[gb, 0, :] = (e / e.sum()).astype(np.float32)
    return out


# revision 22
# speedup vs baseline: 1.0841x; 1.0841x over previous
"""Bass/Trainium2 kernel for nn_Attn_13846974562399.

Reference computation:
    proj   = enc @ W^T + bias          # [S, B, H]
    scores = einsum('bh,sbh->bs', hidden[0], proj)
    attn   = softmax(scores, axis=1)   # -> [B, 1, S]

Algebraic restructure:
    scores[b, s] = q[b] . enc[s, b],   q = hidden[0] @ W
(the hidden.bias term is constant over s and cancels in softmax).  q is
computed on the host in float64; the memory-bound work -- streaming the
encoder tensor and the S*B*H dot-product contraction -- runs on 8
NeuronCores, data-parallel over batch (4 local batches per core).

Precision strategy (the memory-regime key move).  The harness gate is
rel_err < 2e-2.  The device streams the encoder in FP8 E4M3 (8.4 MB per
core, 4x less than fp32) and computes approximate scores s~ = q8 . enc8
with fp32 accumulation; per-score error is ~N(0, 1.2^2).  The host then
(1) ranks each row by s~ and recomputes the top-64 scores EXACTLY
(float64 q . enc from the original fp32 input; 64*H MACs per row =
0.002% of the device FLOPs), and (2) applies softmax over {exact
top-64, fp8 tail}.  Score rows are extremely peaked (std ~32 over 2048
entries), so the tail mass beyond the top-64 is ~1e-13 of the total and
its fp8 distortion is irrelevant: end-to-end rel err measured on
hardware is ~5e-6 (fp16-everywhere gives 6e-3; fp8 without refinement
fails).  Ranking is safe: a true-top entry would need a -10-sigma fp8
error to be misranked out of 64.

Device program (per core).  With the stream at fp8 the DMA is ~26 us
busy (16 SDMA engines, byte-bound) and a single compute engine becomes
the critical path -- a PE-only version measured 259 ns per
[128x1]x[128,512] matmul (~34 us chain; fp8 DoubleRow mode, which would
halve that, crashes this NEFF backend).  So the s-range is SPLIT across
two engines, each with the layout that suits it:

- s in [0, 1408) (68.75%): Tensor engine.  Host layout [b, hc, p, s]
  (h = hc*128+p, contraction dim h on partitions); transfers of
  [128, 1408] fp8 per (b, hc).  Three matmuls per transfer with
  1-column stationary weights accumulate the 8 h-chunks of each score
  group in fp32 PSUM (s-tiles 512/512/384; a PSUM-bank-crossing matmul
  out crashes the backend).  ~23 us.
- s in [1408, 2048) (31.25%): Vector engine (otherwise idle; 8-bit STT
  runs 1 elem/lane/cycle at 0.96 GHz).  Host layout [b, tp, p, t2, h]
  with s = 1408 + (2*tp+t2)*128 + p (s on partitions, t-PAIRS per
  transfer for 2 KB partition lines); one fused scalar_tensor_tensor
  per [128, 1024] chunk multiplies by a replicated q row tile and
  reduces over h into a [128, 20] f32 score tile.  ~24 us.  (Verified
  on HW: fp8 STT inputs with f32 accum_out, rel err 7e-8.)

Transfers are issued in per-batch need order (a DVE pair-chunk feeds
~2.4 us of STT, a PE transfer ~0.73 us of matmul) and each goes to
whichever HWDGE ring has fewer cumulative bytes -- plain alternation
left one ring ~50% heavier inside each batch window and the PE starved
mid-stream for ~6 us waiting on the heavy ring.  Every transfer owns a
private SBUF buffer (~9 MB) so the stream never waits on compute.
~8 warm-up matmuls spin the PE clock from 0.65 toward 2.4 GHz before
real data lands.  Raw fp32 scores ship to the host: PE scores via
PSUM->SBUF copies (split across the scalar AND vector engines -- a
serial 3-copy chain on one engine added ~1.7 us to the tail) then
per-batch DMA (gpsimd SWDGE queue mid-stream -- a dependent trigger on
an in-order HWDGE ring sequencer parks the whole ring -- and the idle
rings for the last batch); DVE scores as one [128, 20] tile at the
end.  No exp/normalization on device -- softmax happens in the host
refinement step.  PSUM: one 3-bank [128, 1536] tile per batch pair,
batch b at base partition 32*(b%2) (PE tile_position allows out base
partitions {0, 32, 64}).
"""

import numpy as np
import ml_dtypes

import concourse.bacc as bacc
import concourse.bass as bass
import concourse.mybir as mybir
import concourse.tile as tile
from concourse.bass_utils import run_bass_kernel_spmd

S, B, H = 2048, 32, 1024
NCORES = 8
BL = B // NCORES          # 4 local batches per core
P = 128                   # SBUF partitions
HC = H // P               # 8 h-chunks per batch
SP = 1536                 # s in [0, SP) on the PE
SD = S - SP               # s in [SP, S) on the DVE
NT = SD // P              # 5 DVE 128-s chunks per batch
NTP = (NT + 1) // 2       # 3 DVE transfers per batch (2+2+1 chunks)
PE_TILES = (512, 512, 512)
F8 = mybir.dt.float8e4
F32 = mybir.dt.float32
NP8 = ml_dtypes.float8_e4m3

WARMUP_MM = 8             # PE pstate ramp matmuls before the stream
TOPK = 64                 # host-refined candidates per row

LAST_RESULTS = None
TRACE = False

_NC = None


def _build_bass():
    nc = bacc.Bacc()
    ence = nc.dram_tensor("ence", [BL, HC, P, SP], F8, kind="ExternalInput")
    # DVE share padded to 6 chunks (3 pair-transfers); chunk 5 is junk.
    encd = nc.dram_tensor("encd", [BL, NTP, P, 2, H], F8, kind="ExternalInput")
    qw = nc.dram_tensor("qw", [P, BL * HC], F8, kind="ExternalInput")
    qrep = nc.dram_tensor("qrep", [BL, P, H], F8, kind="ExternalInput")
    out = nc.dram_tensor("sc", [BL, SP], F32, kind="ExternalOutput")
    outd = nc.dram_tensor("dsc", [P, BL * NT], F32, kind="ExternalOutput")

    mult = mybir.AluOpType.mult

    with tile.TileContext(nc) as tc:
        with (
            tc.tile_pool(name="encp", bufs=BL * HC) as enc_pool,
            tc.tile_pool(name="dvep", bufs=BL * NTP) as dve_pool,
            tc.tile_pool(name="small", bufs=1) as small,
            tc.psum_pool(name="pp", bufs=1) as pp,
        ):
            qw_sb = small.tile([P, BL * HC], F8)
            qrep_sb = [small.tile([P, H], F8, name=f"qr{b}") for b in range(BL)]
            s_sb = small.tile([P, SP], F32)
            dsc = small.tile([P, BL * NT], F32)
            dummy = small.tile([P, 1], F32)
            warm = small.tile([P, 512], F8)

            psum_t = [pp.tile([P, 1536], F32, name=f"ps{g}") for g in range(BL // 2)]

            ence_ap = ence.ap()
            encd_ap = encd.ap()
            out_ap = out.ap()

            ring_bytes = [0, 0]
            rings = [nc.sync, nc.scalar]

            def ring(nbytes):
                i = 0 if ring_bytes[0] <= ring_bytes[1] else 1
                ring_bytes[i] += nbytes
                return rings[i]

            # Per-batch transfer schedule in engine need order; each
            # transfer goes to the ring with fewer cumulative bytes.
            pe_t = {}
            dve_t = {}
            first_issued = False
            ORDER = ["d0", "h0", "h1", "h2", "h3", "d1", "h4", "h5", "h6", "h7"]
            for b in range(BL):
                for item in ORDER:
                    k = int(item[1])
                    if item[0] == "h":
                        et = enc_pool.tile([P, SP], F8)
                        ring(P * SP).dma_start(out=et, in_=ence_ap[b, k])
                        pe_t[(b, k)] = et
                    else:
                        npairs = 2 if k < NTP - 1 else NT - 2 * (NTP - 1)
                        dt = dve_pool.tile([P, 2, H], F8)
                        if npairs == 2:
                            ring(P * 2 * H).dma_start(out=dt, in_=encd_ap[b, k])
                        else:
                            ring(P * H).dma_start(
                                out=dt[:, 0, :], in_=encd_ap[b, k, :, 0]
                            )
                        dve_t[(b, k)] = dt
                    if not first_issued:
                        # The small q tiles slot in right after the first
                        # two stream transfers, balanced across rings.
                        first_issued = True
                        ring_bytes[1] += P * BL * HC
                        nc.scalar.dma_start(out=qw_sb, in_=qw.ap())
                        for bb in range(BL):
                            ring(P * H).dma_start(
                                out=qrep_sb[bb], in_=qrep.ap()[bb]
                            )

            nc.vector.memset(warm, 0.0)
            for _ in range(WARMUP_MM):
                nc.tensor.matmul(
                    out=psum_t[0][64:65, 0:512],
                    lhsT=warm[:, 0:1],
                    rhs=warm,
                    start=True,
                    stop=True,
                )

            for b in range(BL):
                row = slice(32 * (b % 2), 32 * (b % 2) + 1)
                srow = slice(32 * b, 32 * b + 1)
                ps = psum_t[b // 2]
                # DVE chunks for this batch (independent of the PE path).
                for t in range(NT):
                    dt = dve_t[(b, t // 2)]
                    nc.vector.scalar_tensor_tensor(
                        out=dummy.broadcast_to((P, H)),
                        in0=dt[:, t % 2, :],
                        scalar=1.0,
                        in1=qrep_sb[b][:],
                        op0=mult,
                        op1=mult,
                        accum_out=dsc[:, b * NT + t : b * NT + t + 1],
                    )
                # PE h-chunk accumulation.
                for hc in range(HC):
                    c = b * HC + hc
                    et = pe_t[(b, hc)]
                    off = 0
                    for w in PE_TILES:
                        nc.tensor.matmul(
                            out=ps[row, off : off + w],
                            lhsT=qw_sb[:, c : c + 1],
                            rhs=et[:, off : off + w],
                            start=(hc == 0),
                            stop=(hc == HC - 1),
                        )
                        off += w
                # Copies split across scalar+vector so the tail chain is
                # parallel, then one DMA per batch.
                nc.scalar.copy(out=s_sb[srow, 0:512], in_=ps[row, 0:512])
                if b < BL - 1:
                    nc.vector.tensor_scalar_mul(
                        out=s_sb[srow, 512:1024],
                        in0=ps[row, 512:1024],
                        scalar1=1.0,
                    )
                else:
                    nc.scalar.copy(
                        out=s_sb[srow, 512:1024], in_=ps[row, 512:1024]
                    )
                nc.scalar.copy(out=s_sb[srow, 1024:SP], in_=ps[row, 1024:SP])
                if b < BL - 1:
                    nc.gpsimd.dma_start(out=out_ap[b], in_=s_sb[srow, :])
                else:
                    nc.sync.dma_start(out=out_ap[b], in_=s_sb[srow, :])
            nc.scalar.dma_start(out=outd.ap(), in_=dsc)

    nc.compile()
    return nc


def kernel(hidden, encoder_outputs, W, b):
    global _NC, LAST_RESULTS
    hidden = np.asarray(hidden, dtype=np.float32)
    enc = np.asarray(encoder_outputs, dtype=np.float32)
    W = np.asarray(W, dtype=np.float32)

    # q = hidden[0] @ W (fp64 accumulate on host).  The bias adds a per-b
    # constant to the scores, which softmax cancels, so `b` is unused.
    q64 = hidden[0].astype(np.float64) @ W.astype(np.float64)
    q8 = q64.astype(np.float32).astype(NP8)             # [B, H] fp8

    enc8 = enc.astype(NP8)                              # [S, B, H] fp8
    in_maps = []
    for c in range(NCORES):
        sl = enc8[:, BL * c : BL * (c + 1), :]          # [S, BL, H]
        # PE share: [b, h, s<SP] contiguous, h split as (hc, p).
        ence_r = np.ascontiguousarray(sl[:SP].transpose(1, 2, 0)).reshape(
            BL, HC, P, SP
        )
        # DVE share: [b, tp, p, t2, h] with s = SP + (2*tp+t2)*128 + p,
        # padded to 6 chunks (the 6th is junk, never transferred).
        dpad = np.zeros((2 * NTP, P, BL, H), dtype=NP8)
        dpad[:NT] = sl[SP:].reshape(NT, P, BL, H)
        encd_r = np.ascontiguousarray(
            dpad.reshape(NTP, 2, P, BL, H).transpose(3, 0, 2, 1, 4)
        )
        q_c = q8[BL * c : BL * (c + 1)]                 # [BL, H]
        qw_c = np.ascontiguousarray(
            q_c.reshape(BL, HC, P).transpose(2, 0, 1).reshape(P, BL * HC)
        )
        qrep_c = np.ascontiguousarray(
            np.broadcast_to(q_c[:, None, :], (BL, P, H))
        )
        in_maps.append(
            {"ence": ence_r, "encd": encd_r, "qw": qw_c, "qrep": qrep_c}
        )

    if _NC is None:
        _NC = _build_bass()

    LAST_RESULTS = run_bass_kernel_spmd(
        _NC, in_maps, core_ids=list(range(NCORES)), trace=TRACE
    )

    # Host epilogue: reassemble scores, rank rows by the device's fp8
    # scores, recompute the top-64 exactly, softmax over {exact top,
    # fp8 tail}.
    out = np.empty((B, 1, S), dtype=np.float32)
    for c in range(NCORES):
        res = LAST_RESULTS.results[c]
        sc = np.empty((BL, S), dtype=np.float64)
        sc[:, :SP] = res["sc"].astype(np.float64)
        # dsc[p, b*NT + t] -> s = SP + t*128 + p
        d = res["dsc"].astype(np.float64).reshape(P, BL, NT)
        sc[:, SP:] = d.transpose(1, 2, 0).reshape(BL, SD)
        for lb in range(BL):
            gb = BL * c + lb
            idx = np.argpartition(-sc[lb], TOPK)[:TOPK]
            sc[lb, idx] = enc[idx, gb, :].astype(np.float64) @ q64[gb]
            m = sc[lb].max()
            e = np.exp(sc[lb] - m)
            out[gb, 0, :] = (e / e.sum()).astype(np.float32)
    return out


# revision 23
# speedup vs baseline: 1.0911x; 1.0064x over previous
"""Bass/Trainium2 kernel for nn_Attn_13846974562399.

Reference computation:
    proj   = enc @ W^T + bias          # [S, B, H]
    scores = einsum('bh,sbh->bs', hidden[0], proj)
    attn   = softmax(scores, axis=1)   # -> [B, 1, S]

Algebraic restructure:
    scores[b, s] = q[b] . enc[s, b],   q = hidden[0] @ W
(the hidden.bias term is constant over s and cancels in softmax).  q is
computed on the host in float64; the memory-bound work -- streaming the
encoder tensor and the S*B*H dot-product contraction -- runs on 8
NeuronCores, data-parallel over batch (4 local batches per core).

Precision strategy (the memory-regime key move).  The harness gate is
rel_err < 2e-2.  The device streams the encoder in FP8 E4M3 (8.4 MB per
core, 4x less than fp32) and computes approximate scores s~ = q8 . enc8
with fp32 accumulation; per-score error is ~N(0, 1.2^2).  The host then
(1) ranks each row by s~ and recomputes the top-64 scores EXACTLY
(float64 q . enc from the original fp32 input; 64*H MACs per row =
0.002% of the device FLOPs), and (2) applies softmax over {exact
top-64, fp8 tail}.  Score rows are extremely peaked (std ~32 over 2048
entries), so the tail mass beyond the top-64 is ~1e-13 of the total and
its fp8 distortion is irrelevant: end-to-end rel err measured on
hardware is ~5e-6 (fp16-everywhere gives 6e-3; fp8 without refinement
fails).  Ranking is safe: a true-top entry would need a -10-sigma fp8
error to be misranked out of 64.

Device program (per core).  With the stream at fp8 the DMA is ~26 us
busy (16 SDMA engines, byte-bound) and a single compute engine becomes
the critical path -- a PE-only version measured 259 ns per
[128x1]x[128,512] matmul (~34 us chain; fp8 DoubleRow mode, which would
halve that, crashes this NEFF backend).  So the s-range is SPLIT across
two engines, each with the layout that suits it:

- s in [0, 1408) (68.75%): Tensor engine.  Host layout [b, hc, p, s]
  (h = hc*128+p, contraction dim h on partitions); transfers of
  [128, 1408] fp8 per (b, hc).  Three matmuls per transfer with
  1-column stationary weights accumulate the 8 h-chunks of each score
  group in fp32 PSUM (s-tiles 512/512/384; a PSUM-bank-crossing matmul
  out crashes the backend).  ~23 us.
- s in [1408, 2048) (31.25%): Vector engine (otherwise idle; 8-bit STT
  runs 1 elem/lane/cycle at 0.96 GHz).  Host layout [b, tp, p, t2, h]
  with s = 1408 + (2*tp+t2)*128 + p (s on partitions, t-PAIRS per
  transfer for 2 KB partition lines); one fused scalar_tensor_tensor
  per [128, 1024] chunk multiplies by a replicated q row tile and
  reduces over h into a [128, 20] f32 score tile.  ~24 us.  (Verified
  on HW: fp8 STT inputs with f32 accum_out, rel err 7e-8.)

Transfers are issued in per-batch need order (a DVE pair-chunk feeds
~2.4 us of STT, a PE transfer ~0.73 us of matmul) and each goes to
whichever HWDGE ring has fewer cumulative bytes -- plain alternation
left one ring ~50% heavier inside each batch window and the PE starved
mid-stream for ~6 us waiting on the heavy ring.  Every transfer owns a
private SBUF buffer (~9 MB) so the stream never waits on compute.
~8 warm-up matmuls spin the PE clock from 0.65 toward 2.4 GHz before
real data lands.  Raw fp32 scores ship to the host: PE scores via
PSUM->SBUF copies (split across the scalar AND vector engines -- a
serial 3-copy chain on one engine added ~1.7 us to the tail) then
per-batch DMA (gpsimd SWDGE queue mid-stream -- a dependent trigger on
an in-order HWDGE ring sequencer parks the whole ring -- and the idle
rings for the last batch); DVE scores as one [128, 20] tile at the
end.  No exp/normalization on device -- softmax happens in the host
refinement step.  PSUM: one 3-bank [128, 1536] tile per batch pair,
batch b at base partition 32*(b%2) (PE tile_position allows out base
partitions {0, 32, 64}).
"""

import numpy as np
import ml_dtypes

import concourse.bacc as bacc
import concourse.bass as bass
import concourse.mybir as mybir
import concourse.tile as tile
from concourse.bass_utils import run_bass_kernel_spmd

S, B, H = 2048, 32, 1024
NCORES = 8
BL = B // NCORES          # 4 local batches per core
P = 128                   # SBUF partitions
HC = H // P               # 8 h-chunks per batch
SP = 1536                 # s in [0, SP) on the PE
SD = S - SP               # s in [SP, S) on the DVE
NT = SD // P              # 5 DVE 128-s chunks per batch
NTP = (NT + 1) // 2       # 3 DVE transfers per batch (2+2+1 chunks)
PE_TILES = (512, 512, 512)
F8 = mybir.dt.float8e4
F32 = mybir.dt.float32
NP8 = ml_dtypes.float8_e4m3

WARMUP_MM = 8             # PE pstate ramp matmuls before the stream
TOPK = 64                 # host-refined candidates per row

LAST_RESULTS = None
TRACE = False

_NC = None


def _build_bass():
    nc = bacc.Bacc()
    ence = nc.dram_tensor("ence", [BL, HC // 2, P, 2, SP], F8, kind="ExternalInput")
    # DVE share padded to 6 chunks (3 pair-transfers); chunk 5 is junk.
    encd = nc.dram_tensor("encd", [BL, NTP, P, 2, H], F8, kind="ExternalInput")
    qw = nc.dram_tensor("qw", [P, BL * HC], F8, kind="ExternalInput")
    qrep = nc.dram_tensor("qrep", [BL, P, H], F8, kind="ExternalInput")
    out = nc.dram_tensor("sc", [BL, SP], F32, kind="ExternalOutput")
    outd = nc.dram_tensor("dsc", [P, BL * NT], F32, kind="ExternalOutput")

    mult = mybir.AluOpType.mult

    with tile.TileContext(nc) as tc:
        with (
            tc.tile_pool(name="encp", bufs=BL * HC // 2) as enc_pool,
            tc.tile_pool(name="dvep", bufs=BL * NTP) as dve_pool,
            tc.tile_pool(name="small", bufs=1) as small,
            tc.psum_pool(name="pp", bufs=1) as pp,
        ):
            qw_sb = small.tile([P, BL * HC], F8)
            qrep_sb = [small.tile([P, H], F8, name=f"qr{b}") for b in range(BL)]
            s_sb = small.tile([P, SP], F32)
            dsc = small.tile([P, BL * NT], F32)
            dummy = small.tile([P, 1], F32)
            warm = small.tile([P, 512], F8)

            psum_t = [pp.tile([P, 1536], F32, name=f"ps{g}") for g in range(BL // 2)]

            ence_ap = ence.ap()
            encd_ap = encd.ap()
            out_ap = out.ap()

            ring_bytes = [0, 0]
            rings = [nc.sync, nc.scalar]

            def ring(nbytes):
                i = 0 if ring_bytes[0] <= ring_bytes[1] else 1
                ring_bytes[i] += nbytes
                return rings[i]

            # Per-batch transfer schedule in engine need order; each
            # transfer goes to the ring with fewer cumulative bytes.
            pe_t = {}
            dve_t = {}
            first_issued = False
            ORDER = ["d0", "h0", "h1", "d1", "h2", "h3"]
            for b in range(BL):
                for item in ORDER:
                    k = int(item[1])
                    if item[0] == "h":
                        et = enc_pool.tile([P, 2, SP], F8)
                        ring(P * 2 * SP).dma_start(out=et, in_=ence_ap[b, k])
                        pe_t[(b, k)] = et
                    else:
                        npairs = 2 if k < NTP - 1 else NT - 2 * (NTP - 1)
                        dt = dve_pool.tile([P, 2, H], F8)
                        if npairs == 2:
                            ring(P * 2 * H).dma_start(out=dt, in_=encd_ap[b, k])
                        else:
                            ring(P * H).dma_start(
                                out=dt[:, 0, :], in_=encd_ap[b, k, :, 0]
                            )
                        dve_t[(b, k)] = dt
                    if not first_issued:
                        # The small q tiles slot in right after the first
                        # two stream transfers, balanced across rings.
                        first_issued = True
                        ring_bytes[1] += P * BL * HC
                        nc.scalar.dma_start(out=qw_sb, in_=qw.ap())
                        for bb in range(BL):
                            ring(P * H).dma_start(
                                out=qrep_sb[bb], in_=qrep.ap()[bb]
                            )

            nc.vector.memset(warm, 0.0)
            for _ in range(WARMUP_MM):
                nc.tensor.matmul(
                    out=psum_t[0][64:65, 0:512],
                    lhsT=warm[:, 0:1],
                    rhs=warm,
                    start=True,
                    stop=True,
                )

            for b in range(BL):
                row = slice(32 * (b % 2), 32 * (b % 2) + 1)
                srow = slice(32 * b, 32 * b + 1)
                ps = psum_t[b // 2]
                # DVE chunks for this batch (independent of the PE path).
                for t in range(NT):
                    dt = dve_t[(b, t // 2)]
                    nc.vector.scalar_tensor_tensor(
                        out=dummy.broadcast_to((P, H)),
                        in0=dt[:, t % 2, :],
                        scalar=1.0,
                        in1=qrep_sb[b][:],
                        op0=mult,
                        op1=mult,
                        accum_out=dsc[:, b * NT + t : b * NT + t + 1],
                    )
                # PE h-chunk accumulation (pair transfers, 2 hc each).
                for hc in range(HC):
                    c = b * HC + hc
                    et = pe_t[(b, hc // 2)]
                    off = 0
                    for w in PE_TILES:
                        nc.tensor.matmul(
                            out=ps[row, off : off + w],
                            lhsT=qw_sb[:, c : c + 1],
                            rhs=et[:, hc % 2, off : off + w],
                            start=(hc == 0),
                            stop=(hc == HC - 1),
                        )
                        off += w
                # Copies split across scalar+vector so the tail chain is
                # parallel, then one DMA per batch.
                nc.scalar.copy(out=s_sb[srow, 0:512], in_=ps[row, 0:512])
                nc.vector.tensor_scalar_mul(
                    out=s_sb[srow, 512:1024],
                    in0=ps[row, 512:1024],
                    scalar1=1.0,
                )
                nc.scalar.copy(out=s_sb[srow, 1024:SP], in_=ps[row, 1024:SP])
                if b < BL - 1:
                    nc.gpsimd.dma_start(out=out_ap[b], in_=s_sb[srow, :])
                else:
                    nc.sync.dma_start(out=out_ap[b], in_=s_sb[srow, :])
            nc.scalar.dma_start(out=outd.ap(), in_=dsc)

    nc.compile()
    return nc


def kernel(hidden, encoder_outputs, W, b):
    global _NC, LAST_RESULTS
    hidden = np.asarray(hidden, dtype=np.float32)
    enc = np.asarray(encoder_outputs, dtype=np.float32)
    W = np.asarray(W, dtype=np.float32)

    # q = hidden[0] @ W (fp64 accumulate on host).  The bias adds a per-b
    # constant to the scores, which softmax cancels, so `b` is unused.
    q64 = hidden[0].astype(np.float64) @ W.astype(np.float64)
    q8 = q64.astype(np.float32).astype(NP8)             # [B, H] fp8

    enc8 = enc.astype(NP8)                              # [S, B, H] fp8
    in_maps = []
    for c in range(NCORES):
        sl = enc8[:, BL * c : BL * (c + 1), :]          # [S, BL, H]
        # PE share: [b, h, s<SP] contiguous, h split as (hc, p).
        # [b, hcp, p, j, s] with h = hcp*256 + j*128 + p: pair transfers
        # with 3 KB partition lines.
        ence_r = np.ascontiguousarray(
            sl[:SP].reshape(SP, BL, HC // 2, 2, P).transpose(1, 2, 4, 3, 0)
        )
        # DVE share: [b, tp, p, t2, h] with s = SP + (2*tp+t2)*128 + p,
        # padded to 6 chunks (the 6th is junk, never transferred).
        dpad = np.zeros((2 * NTP, P, BL, H), dtype=NP8)
        dpad[:NT] = sl[SP:].reshape(NT, P, BL, H)
        encd_r = np.ascontiguousarray(
            dpad.reshape(NTP, 2, P, BL, H).transpose(3, 0, 2, 1, 4)
        )
        q_c = q8[BL * c : BL * (c + 1)]                 # [BL, H]
        qw_c = np.ascontiguousarray(
            q_c.reshape(BL, HC, P).transpose(2, 0, 1).reshape(P, BL * HC)
        )
        qrep_c = np.ascontiguousarray(
            np.broadcast_to(q_c[:, None, :], (BL, P, H))
        )
        in_maps.append(
            {"ence": ence_r, "encd": encd_r, "qw": qw_c, "qrep": qrep_c}
        )

    if _NC is None:
        _NC = _build_bass()

    LAST_RESULTS = run_bass_kernel_spmd(
        _NC, in_maps, core_ids=list(range(NCORES)), trace=TRACE
    )

    # Host epilogue: reassemble scores, rank rows by the device's fp8
    # scores, recompute the top-64 exactly, softmax over {exact top,
    # fp8 tail}.
    out = np.empty((B, 1, S), dtype=np.float32)
    for c in range(NCORES):
        res = LAST_RESULTS.results[c]
        sc = np.empty((BL, S), dtype=np.float64)
        sc[:, :SP] = res["sc"].astype(np.float64)
        # dsc[p, b*NT + t] -> s = SP + t*128 + p
        d = res["dsc"].astype(np.float64).reshape(P, BL, NT)
        sc[:, SP:] = d.transpose(1, 2, 0).reshape(BL, SD)
        for lb in range(BL):
            gb = BL * c + lb
            idx = np.argpartition(-sc[lb], TOPK)[:TOPK]
            sc[lb, idx] = enc[idx, gb, :].astype(np.float64) @ q64[gb]
            m = sc[lb].max()
            e = np.exp(sc[lb] - m)
            out[gb, 0, :] = (e / e.sum()).astype(np.float32)
    return out
